# revision 6
# baseline (speedup 1.0000x reference)
"""Trainium2 Bass kernel for causal top-K (K=8) similarity message passing.

Math per batch b (reference):
  gate  = sigmoid(x @ w_gate + b_gate)                      (B,T)
  S     = x @ x^T, causal-masked to NEG=-1e30               (B,T,T)
  top-8 per row -> selected neighbour set, count=min(i+1,8)
  msg   = mean of selected x rows
  blend = mix*x + (1-mix)*msg
  out   = gate * gelu(blend*gain + bias) * (softplus(log_scale)+0.01)

Sharding: 8 cores = 4 batches x 2 query-parity shards. Core c handles
batch b=c>>1, parity p=c&1, processing query chunks Qg = 2t+p
(t=0..15) of 128 rows each. Every core runs a uniform SPMD program;
all parity dependence is carried in per-core input data.

The full working set lives in SBUF in fp16 (three host-prepared
layouts of x: xt transposed for scores, xq query stationaries, xg
gain-scaled values for aggregation). Loads are split by key quartile
(low keys first) so each range's reload overlaps the previous
iteration's tail and unblocks the early tiles of the next one.

Per query tile t (Lc = 2t+2 key chunks of 128; for p=0 the last chunk
is fully masked so both parities run the identical program):
  scores  = xq[t]^T @ xt (PE fp16, f32 PSUM) -> SBUF with causal masks
  v8      = max8(scores), tau = v8[:,7]      (DVE top-8 instruction)
  sel     = (scores >= tau) as fp16, fixed up for tile 0; the diagonal
            gets mix*count/(1-mix) added so the blend's mix*x term
            rides the aggregation matmul
  pm      = eta*bias outer product + sum_c transpose(sel chunk) @ xg
  y[t]    = sigmoid(gate_lin)*scale * gelu(pm * (1-mix)/count)
"""

import os
import sys

for _p in ("/opt/trn_rl_repo", os.path.expanduser("~/.axon_site/_ro/trn_rl_repo")):
    if os.path.isdir(_p) and _p not in sys.path:
        sys.path.insert(0, _p)
        break

import numpy as np

import concourse.bacc as bacc
import concourse.mybir as mybir
from concourse import masks
from concourse.tile import TileContext
from concourse.bass_utils import run_bass_kernel_spmd

F32 = mybir.dt.float32
F16 = mybir.dt.float16
AF = mybir.ActivationFunctionType
ALU = mybir.AluOpType
NEG = np.float32(-1e30)

T = 4096
D = 1024
DC = 8          # D // 128
NQT = 16        # query tiles per core (T // 256)
N_CORES = 8

_prog_cache = {}


def build_program(reps=1, stage=5):
    """Build + bass-compile the uniform per-core program.

    reps: static repetitions of the whole computation (for HW timing).
    stage: debug cutoff (5 = full kernel).
    """
    key = (reps, stage)
    if key in _prog_cache:
        return _prog_cache[key]

    nc = bacc.Bacc(trn_type="TRN2", target_bir_lowering=False, debug=False,
                   num_devices=N_CORES, dynamic_dma_scratch_size=512)

    xt_in = nc.dram_tensor("xt", [128, 4, DC, T // 4], F16,
                           kind="ExternalInput").ap()
    xq_in = nc.dram_tensor("xq", [128, NQT, DC, 128], F16,
                           kind="ExternalInput").ap()
    xg_in = nc.dram_tensor("xg", [128, T // 128, D], F16,
                           kind="ExternalInput").ap()
    qmask_in = nc.dram_tensor("qmask", [128, 256], F32, kind="ExternalInput").ap()
    smask_in = nc.dram_tensor("smask", [128, 256], F16, kind="ExternalInput").ap()
    dmask_in = nc.dram_tensor("dmask", [128, 256], F16, kind="ExternalInput").ap()
    dmask0_in = nc.dram_tensor("dmask0", [128, 256], F16, kind="ExternalInput").ap()
    recip_in = nc.dram_tensor("recipc", [128, NQT], F32, kind="ExternalInput").ap()
    eta_in = nc.dram_tensor("eta", [1, NQT, 128], F16, kind="ExternalInput").ap()
    biasr_in = nc.dram_tensor("biasr", [1, D], F16, kind="ExternalInput").ap()
    wg_in = nc.dram_tensor("wg", [128, DC], F16, kind="ExternalInput").ap()
    sc_in = nc.dram_tensor("sc", [128, 2], F32, kind="ExternalInput").ap()
    y_out = nc.dram_tensor("y", [NQT, 128, D], F16, kind="ExternalOutput").ap()

    from contextlib import ExitStack

    with TileContext(nc) as tc, ExitStack() as ctx:
        cpool = ctx.enter_context(tc.tile_pool(name="consts", bufs=1))
        xTp = ctx.enter_context(tc.tile_pool(name="xTp", bufs=1))
        xGp = ctx.enter_context(tc.tile_pool(name="xGp", bufs=1))
        xQp = ctx.enter_context(tc.tile_pool(name="xQp", bufs=1))
        Sp = ctx.enter_context(tc.tile_pool(name="Sp", bufs=2))
        selp = ctx.enter_context(tc.tile_pool(name="selp", bufs=2))
        stp = ctx.enter_context(tc.tile_pool(name="stp", bufs=3))
        msgp = ctx.enter_context(tc.tile_pool(name="msgp", bufs=2))
        smallp = ctx.enter_context(tc.tile_pool(name="smallp", bufs=2))
        ps_s = ctx.enter_context(tc.tile_pool(name="ps_s", bufs=2, space="PSUM"))
        ps_t = ctx.enter_context(tc.tile_pool(name="ps_t", bufs=2, space="PSUM"))
        ps_m = ctx.enter_context(tc.tile_pool(name="ps_m", bufs=1, space="PSUM"))
        ps_g = ctx.enter_context(tc.tile_pool(name="ps_g", bufs=1, space="PSUM"))

        qmask = cpool.tile([128, 256], F32)
        nc.sync.dma_start(out=qmask[:], in_=qmask_in[:])
        smask = cpool.tile([128, 256], F16)
        nc.sync.dma_start(out=smask[:], in_=smask_in[:])
        dmask = cpool.tile([128, 256], F16)
        nc.sync.dma_start(out=dmask[:], in_=dmask_in[:])
        dmask0 = cpool.tile([128, 256], F16)
        nc.sync.dma_start(out=dmask0[:], in_=dmask0_in[:])
        recip = cpool.tile([128, NQT], F32)
        nc.sync.dma_start(out=recip[:], in_=recip_in[:])
        eta = cpool.tile([1, NQT, 128], F16)
        nc.sync.dma_start(out=eta[:], in_=eta_in[:])
        biasr = cpool.tile([1, D], F16)
        nc.sync.dma_start(out=biasr[:], in_=biasr_in[:])
        wg = cpool.tile([128, DC], F16)
        nc.sync.dma_start(out=wg[:], in_=wg_in[:])
        sc = cpool.tile([128, 2], F32)
        nc.sync.dma_start(out=sc[:], in_=sc_in[:])
        ident32 = cpool.tile([128, 128], F32)
        masks.make_identity(nc, ident32[:])
        identH = cpool.tile([128, 128], F16)
        nc.scalar.copy(identH[:], ident32[:])

        for _rep in range(reps):
            # ---- load the working set, split by key quartile (low first)
            # so each range's load overlaps the previous iteration's tail
            # and unblocks the early tiles of this one ----
            xT = xTp.tile([128, 4, DC, T // 4], F16, tag="xT", name="xT")
            xQ = xQp.tile([128, NQT, DC, 128], F16, tag="xQ", name="xQ")
            xG = xGp.tile([128, T // 128, D], F16, tag="xG", name="xG")
            for qtr in range(4):
                nc.sync.dma_start(out=xQ[:, 4 * qtr:4 * qtr + 4],
                                  in_=xq_in[:, 4 * qtr:4 * qtr + 4])
                nc.sync.dma_start(out=xT[:, qtr], in_=xt_in[:, qtr])
                nc.sync.dma_start(out=xG[:, 8 * qtr:8 * qtr + 8, :],
                                  in_=xg_in[:, 8 * qtr:8 * qtr + 8, :])

            if stage <= 1:
                dbg = msgp.tile([128, D], F16, tag="msg", name="dbg")
                nc.vector.tensor_copy(dbg[:], xG[:, 0])
                nc.sync.dma_start(out=y_out[0], in_=dbg[:])
                continue

            for t in range(NQT):
                Lc = 2 * t + 2
                Lk = Lc * 128

                # ---- scores: S[q, k] = sum_d x[q,d] x[k,d] ----
                S = Sp.tile([128, T], F32, tag="S", name="S")
                nblk = (Lk + 511) // 512
                for blk in range(nblk):
                    w = min(512, Lk - blk * 512)
                    ps = ps_s.tile([128, 512], F32, tag="ps", name="ps")
                    qtr, off = (blk * 512) // 1024, (blk * 512) % 1024
                    for dc in range(DC):
                        nc.tensor.matmul(ps[:, :w], xQ[:, t, dc],
                                         xT[:, qtr, dc, off:off + w],
                                         start=(dc == 0), stop=(dc == DC - 1))
                    lo = blk * 512
                    plain_w = w if blk < nblk - 1 else w - 256
                    if plain_w > 0:
                        nc.scalar.copy(S[:, lo:lo + plain_w], ps[:, :plain_w])
                    if blk == nblk - 1:
                        nc.vector.tensor_add(S[:, Lk - 256:Lk],
                                             ps[:, w - 256:w], qmask[:])

                if stage <= 2:
                    dbg2 = msgp.tile([128, D], F16, tag="msg", name="dbg2")
                    nc.vector.tensor_copy(dbg2[:], S[:, 0:D])
                    nc.sync.dma_start(out=y_out[t], in_=dbg2[:])
                    continue

                # ---- top-8 threshold -> fp16 selection weights ----
                v8 = smallp.tile([128, 8], F32, tag="v8", name="v8")
                nc.vector.max(out=v8[:], in_=S[:, :Lk])
                sel = selp.tile([128, T], F16, tag="sel", name="sel")
                nc.vector.tensor_scalar(sel[:, :Lk], S[:, :Lk], v8[:, 7:8],
                                        None, op0=ALU.is_ge)
                if t == 0:
                    nc.vector.tensor_mul(sel[:, :256], sel[:, :256], smask[:])
                dm = dmask0 if t == 0 else dmask
                nc.vector.tensor_add(sel[:, Lk - 256:Lk], sel[:, Lk - 256:Lk],
                                     dm[:])

                if stage <= 3:
                    dbg3 = msgp.tile([128, D], F16, tag="msg", name="dbg3")
                    nc.vector.tensor_copy(dbg3[:], sel[:, 0:D])
                    nc.sync.dma_start(out=y_out[t], in_=dbg3[:])
                    continue

                # ---- gate ----
                pg = ps_g.tile([128, 1], F32, tag="pg", name="pg")
                for dc in range(DC):
                    nc.tensor.matmul(pg[:], xQ[:, t, dc], wg[:, dc:dc + 1],
                                     start=(dc == 0), stop=(dc == DC - 1))
                gate = smallp.tile([128, 1], F32, tag="gate", name="gate")
                nc.scalar.activation(gate[:], pg[:], AF.Sigmoid,
                                     bias=sc[:, 0:1], scale=1.0)
                nc.vector.tensor_mul(gate[:], gate[:], sc[:, 1:2])

                # ---- aggregation: pm = eta*bias + sel^T @ (x*gain) ----
                pm = ps_m.tile([128, D], F32, tag="pm", name="pm")
                for h in (0, 1):
                    nc.tensor.matmul(pm[:, h * 512:(h + 1) * 512],
                                     eta[0:1, t], biasr[0:1, h * 512:(h + 1) * 512],
                                     start=True, stop=False)
                for c in range(Lc):
                    pt = ps_t.tile([128, 128], F16, tag="pt", name="pt")
                    nc.tensor.transpose(pt[:], sel[:, c * 128:(c + 1) * 128],
                                        identH[:])
                    sT = stp.tile([128, 128], F16, tag="sT", name="sT")
                    nc.vector.tensor_copy(sT[:], pt[:])
                    for h in (0, 1):
                        nc.tensor.matmul(pm[:, h * 512:(h + 1) * 512], sT[:],
                                         xG[:, c, h * 512:(h + 1) * 512],
                                         start=False, stop=(c == Lc - 1))

                if stage <= 4:
                    dbg4 = msgp.tile([128, D], F16, tag="msg", name="dbg4")
                    nc.scalar.copy(dbg4[:], pm[:])
                    nc.sync.dma_start(out=y_out[t], in_=dbg4[:])
                    continue

                # ---- tail: y[t] = gate*scale * gelu(pm * recip) ----
                msg = msgp.tile([128, D], F16, tag="msg", name="msg")
                nc.scalar.activation(msg[:], pm[:], AF.Gelu,
                                     scale=recip[:, t:t + 1])
                nc.vector.tensor_scalar(msg[:], msg[:], gate[:, 0:1], None,
                                        op0=ALU.mult)
                nc.sync.dma_start(out=y_out[t], in_=msg[:])

    nc.compile()
    _prog_cache[key] = nc
    return nc


def host_inputs(xb, p, mix, scale, b_gate, w_gate, gain, bias):
    """Per-core input arrays for batch slice xb (T,D) and parity p."""
    f32, f16 = np.float32, np.float16
    xb = np.ascontiguousarray(xb, f32)

    xt = xb.T.reshape(DC, 128, T).transpose(1, 0, 2)          # [dp, dc, k]
    xq = xt.reshape(128, DC, T // 128, 128)[:, :, p::2, :] \
           .transpose(0, 2, 1, 3)                             # [dp, t, dc, q]
    xg = (xb * np.asarray(gain, f32)[None, :]) \
        .reshape(T // 128, 128, D).transpose(1, 0, 2)         # [kp, c, d]

    r = np.arange(128)
    tri_add = np.where(r[None, :] <= r[:, None], f32(0), NEG).astype(f32)
    tri01 = (r[None, :] <= r[:, None]).astype(f32)
    qmask = np.zeros((128, 256), f32)
    smask = np.zeros((128, 256), f32)
    if p == 0:
        qmask[:, :128] = tri_add
        qmask[:, 128:] = NEG
        smask[:, :128] = tri01
    else:
        qmask[:, 128:] = tri_add
        smask[:, :128] = 1.0
        smask[:, 128:] = tri01

    # counts: count(t, q) = min((2t+p)*128 + q + 1, 8)
    g_row = (2 * np.arange(NQT)[:, None] + p) * 128 + r[None, :]
    counts = np.minimum(g_row + 1, 8).astype(f32)

    dmask = np.zeros((128, 256), f32)
    dmask0 = np.zeros((128, 256), f32)
    half = 0 if p == 0 else 128
    mixfac_n = mix * 8.0 / (1.0 - mix)
    mixfac_0 = mix * counts[0] / (1.0 - mix)
    dmask[r, half + r] = mixfac_n
    dmask0[r, half + r] = mixfac_0

    recipc = np.ascontiguousarray(((1.0 - mix) / counts).T)     # (128, NQT)
    eta = np.ascontiguousarray((counts / (1.0 - mix))[None])    # (1, NQT, 128)

    wg = np.ascontiguousarray(np.asarray(w_gate, f32).reshape(DC, 128).T)
    sc_arr = np.zeros((128, 2), f32)
    sc_arr[:, 0] = b_gate
    sc_arr[:, 1] = scale
    return {
        "xt": np.ascontiguousarray(
            xt.reshape(128, DC, 4, T // 4).transpose(0, 2, 1, 3), f16),
        "xq": np.ascontiguousarray(xq, f16),
        "xg": np.ascontiguousarray(xg, f16),
        "qmask": qmask,
        "smask": smask.astype(f16),
        "dmask": dmask.astype(f16),
        "dmask0": dmask0.astype(f16),
        "recipc": recipc.astype(f32),
        "eta": eta.astype(f16),
        "biasr": np.ascontiguousarray(np.asarray(bias, f32)[None, :]).astype(f16),
        "wg": wg.astype(f16),
        "sc": sc_arr,
    }


def _derive_params(x, b_gate, log_mix, log_scale):
    x = np.asarray(x, np.float32)
    mix = float(1.0 / (1.0 + np.exp(-np.float64(log_mix))))
    scale = float(np.logaddexp(0.0, np.float64(log_scale)) + 0.01)
    b_gate_f = float(np.asarray(b_gate, np.float64))
    return x, mix, scale, b_gate_f


def make_in_maps(x, w_gate, b_gate, gain, bias, log_mix, log_scale):
    x, mix, scale, b_gate_f = _derive_params(x, b_gate, log_mix, log_scale)
    B = x.shape[0]
    in_maps = []
    for core in range(N_CORES):
        b, p = core >> 1, core & 1
        in_maps.append(host_inputs(x[b % B], p, mix, scale, b_gate_f,
                                   w_gate, gain, bias))
    return in_maps


def assemble_output(per_core_y, B):
    """per_core_y: list of 8 arrays [NQT, 128, D] (fp16) -> (B,T,D) f32."""
    out = np.empty((B, T, D), np.float32)
    for core in range(N_CORES):
        b, p = core >> 1, core & 1
        if b >= B:
            continue
        out[b].reshape(T // 128, 128, D)[p::2] = \
            np.asarray(per_core_y[core]).astype(np.float32)
    return out


def run_cores(x, w_gate, b_gate, gain, bias, log_mix, log_scale,
              reps=1, stage=5):
    """Run the SPMD program over all 8 cores; returns (B,T,D) output."""
    nc = build_program(reps=reps, stage=stage)
    in_maps = make_in_maps(x, w_gate, b_gate, gain, bias, log_mix, log_scale)
    res = run_bass_kernel_spmd(nc, in_maps, list(range(N_CORES)))
    return assemble_output([res.results[c]["y"] for c in range(N_CORES)],
                           np.asarray(x).shape[0])


# ---------------------------------------------------------------------------
# Fast execution path: device-resident inputs + repeated sharded-jit calls.
# Mirrors concourse.bass2jax.run_bass_via_pjrt but keeps the uploaded inputs
# alive so repeated calls skip the host->device transfer. Used for HW timing.
# ---------------------------------------------------------------------------

def make_runner(nc, in_maps, n_cores=N_CORES):
    import jax
    from jax.sharding import Mesh, PartitionSpec, NamedSharding
    try:
        from jax.experimental.shard_map import shard_map
        _sm_kw = {"check_rep": False}
    except ImportError:
        from jax import shard_map
        _sm_kw = {"check_vma": False}
    from concourse import bass2jax

    bass2jax.install_neuronx_cc_hook()
    if nc.dbg_addr is not None:
        in_maps = [{**m, nc.dbg_addr.name: np.zeros((1, 2), np.uint32)}
                   for m in in_maps]
    partition_name = (nc.partition_id_tensor.name
                      if nc.partition_id_tensor else None)
    in_names, out_names, out_avals, zero_outs = [], [], [], []
    for alloc in nc.m.functions[0].allocations:
        if not isinstance(alloc, mybir.MemoryLocationSet):
            continue
        name = alloc.memorylocations[0].name
        if alloc.kind == "ExternalInput":
            if name != partition_name:
                in_names.append(name)
        elif alloc.kind == "ExternalOutput":
            shape = tuple(alloc.tensor_shape)
            dtype = mybir.dt.np(alloc.dtype)
            out_names.append(name)
            out_avals.append(jax.core.ShapedArray(shape, dtype))
            zero_outs.append(np.zeros(shape, dtype))
    n_params = len(in_names)
    n_outs = len(out_avals)
    all_in_names = list(in_names) + list(out_names)
    if partition_name is not None:
        all_in_names.append(partition_name)

    def _body(*args):
        operands = list(args)
        if partition_name is not None:
            operands.append(bass2jax.partition_id_tensor())
        outs = bass2jax._bass_exec_p.bind(
            *operands,
            out_avals=tuple(out_avals),
            in_names=tuple(all_in_names),
            out_names=tuple(out_names),
            lowering_input_output_aliases=(),
            sim_require_finite=True,
            sim_require_nnan=True,
            nc=nc,
        )
        return tuple(outs)

    devices = jax.devices()[:n_cores]
    mesh = Mesh(np.asarray(devices), ("core",))
    in_specs = (PartitionSpec("core"),) * (n_params + n_outs)
    out_specs = (PartitionSpec("core"),) * len(out_names)
    sharded = jax.jit(
        shard_map(_body, mesh=mesh, in_specs=in_specs, out_specs=out_specs,
                  **_sm_kw),
        keep_unused=True)

    sh = NamedSharding(mesh, PartitionSpec("core"))
    concat_in = [
        jax.device_put(
            np.concatenate([np.asarray(in_maps[c][nm])
                            for c in range(n_cores)], axis=0), sh)
        for nm in in_names
    ]
    concat_zeros = [
        jax.device_put(
            np.zeros((n_cores * z.shape[0], *z.shape[1:]), z.dtype), sh)
        for z in zero_outs
    ]
    for a in concat_in + concat_zeros:
        a.block_until_ready()

    def run_async():
        return sharded(*concat_in, *concat_zeros)

    def run():
        outs = run_async()
        for o in outs:
            o.block_until_ready()
        return outs

    run.run_async = run_async
    run.out_names = out_names
    return run


def measure_hw_ns(in_maps, reps_pair=(1, 17), M_pair=(1, 33), samples=8):
    """Per-iteration HW time via async-pipelined static-reps delta.

    For each program (reps=a, reps=b) measure the marginal wall time of one
    additional pipelined execution (M=1 vs M=33 back-to-back submissions),
    then per-rep = (marg_b - marg_a) / (b - a). This subtracts both the
    per-call RPC overhead and the per-invocation NEFF overhead, mirroring
    the (t(iters=R)-t(iters=1))/(R-1) methodology.
    """
    import time
    margs = {}
    for reps in reps_pair:
        nc = build_program(reps=reps)
        run = make_runner(nc, in_maps)
        run()
        run()
        best = {}
        for M in M_pair:
            b = 1e9
            for _ in range(samples):
                t0 = time.time()
                outs = None
                for _ in range(M):
                    outs = run.run_async()
                for o in outs:
                    o.block_until_ready()
                b = min(b, time.time() - t0)
            best[M] = b
        margs[reps] = (best[M_pair[1]] - best[M_pair[0]]) / (M_pair[1] - M_pair[0])
    a, b = reps_pair
    per_rep = (margs[b] - margs[a]) / (b - a)
    return per_rep * 1e9, margs


def kernel(x, w_gate, b_gate, gain, bias, log_mix, log_scale, K):
    assert int(K) == 8, "kernel is specialized for K=8"
    return run_cores(x, w_gate, b_gate, gain, bias, log_mix, log_scale)
